# revision 21
# baseline (speedup 1.0000x reference)
"""Causal self-attention (B=4, T=2048, C=2048, H=16, rope) on 8 trn2 cores.

Sharding: core c handles batch b = c//2 and head-group g = c%2 (8 heads).

All-bf16 datapath (PSUM accumulation in f32): x is host-cast to bf16 and
kept fully resident in SBUF; q/k/v never leave SBUF (no DRAM round-trip).
Rolling per-head schedule: attention of head h is emission-interleaved
with the qkv GEMMs of head h+1, so the ACT-bound softmax hides under the
PE-bound qkv and each head's y AllGather fires at head cadence.

  qkv (per head, per 512-t quarter): ps = W_tile^T x (16 bf16 matmuls,
    N=512); v is cast bf16 + PE-transposed into va[t,d] with a ones
    column for the softmax denominator (transposes deferred one unit so
    the ACT cast never stalls the PE); q/k get rope on DVE (sign-vector
    trick) and land bf16 in a persistent per-head SBUF slot.
  attention (per head, per 512-q chunk): scoresT = k_tile^T q into
    single-bank PSUM tiles (4 bufs) so exp on ACT pipelines 4 deep,
    causal mask multiply on Pool, attn@V with the ones column so the
    denominator falls out of the same matmul, reciprocal normalize,
    PE-transpose y to [d, t] (deferred one q-tile).
  epilogue (per head): yts -> DRAM on the Pool queue, then pairwise
    AllGather (8 small collectives at head cadence).
  proj: out[t, f-half] accumulated in four passes by collective arrival
    time (heads 0-3 / 4-5 / 6 / 7) with bf16 partials resident in SBUF;
    the first pass is interleaved into the last head's ACT-bound
    attention, and the late pair-core collectives never stall the PE.
"""
import sys

sys.path.insert(0, "/opt/trn_rl_repo")

import numpy as np
import ml_dtypes

import concourse.bass as bass
import concourse.tile as tile
from concourse import bacc, mybir
from concourse import bass_utils

F32 = mybir.dt.float32
BF16 = mybir.dt.bfloat16
AF = mybir.ActivationFunctionType
ALU = mybir.AluOpType
BF16NP = ml_dtypes.bfloat16

B, T, C = 4, 2048, 2048
NH, D = 16, 128
HL = 8              # heads per core
NCT = C // 128      # 16 c-tiles
NTT = T // 128      # 16 t-tiles
SCALE = 1.0 / np.sqrt(D)
RG = [[0, 1], [2, 3], [4, 5], [6, 7]]


def _interleave(units_a, units_b):
    """Round-robin emit closures from two lists, proportionally."""
    na, nb = len(units_a), len(units_b)
    ia = ib = 0
    while ia < na or ib < nb:
        if ib >= nb or (ia < na and ia * nb <= ib * na):
            units_a[ia]()
            ia += 1
        else:
            units_b[ib]()
            ib += 1


def _build():
    nc = bacc.Bacc("TRN2", target_bir_lowering=False, debug=False, num_devices=8)
    xT = nc.dram_tensor("xT", [128, NCT, T], BF16, kind="ExternalInput").ap()
    Wall = nc.dram_tensor("Wall", [24, 128, C], BF16, kind="ExternalInput").ap()
    WpT = nc.dram_tensor("WpT", [128, NCT, C // 2], BF16, kind="ExternalInput").ap()
    cos2 = nc.dram_tensor("cos2", [128, T], F32, kind="ExternalInput").ap()
    sin1 = nc.dram_tensor("sin1", [64, T], F32, kind="ExternalInput").ap()
    sgn = nc.dram_tensor("sgn", [128, 1], F32, kind="ExternalInput").ap()
    mask4 = nc.dram_tensor("mask4", [128, 4, 512], BF16, kind="ExternalInput").ap()
    ident = nc.dram_tensor("ident", [128, 128], BF16, kind="ExternalInput").ap()
    out = nc.dram_tensor("out", [T, C // 2], F32, kind="ExternalOutput").ap()

    with tile.TileContext(nc) as tc:
        with tc.tile_pool(name="dram", bufs=1, space="DRAM") as dram, \
             tc.tile_pool(name="const", bufs=1) as cpool:
            yg_in = [dram.tile([128, T], BF16, name=f"yg_in{h}")
                     for h in range(HL)]
            yg_out = [dram.tile([2, 128, T], BF16, name=f"yg_out{h}")
                      for h in range(HL)]

            # stack order matters: pools released early must be on top
            yts_pool = tc.alloc_tile_pool(name="ytsp", bufs=1)
            qk_pool = tc.alloc_tile_pool(name="qkp", bufs=1)
            va_pool = tc.alloc_tile_pool(name="vap", bufs=1)
            eb_pool = tc.alloc_tile_pool(name="ebp", bufs=20)
            yn_pool = tc.alloc_tile_pool(name="ynp", bufs=3)
            rc_pool = tc.alloc_tile_pool(name="rcp", bufs=3)
            x_pool = tc.alloc_tile_pool(name="xp", bufs=1)
            w_pool = tc.alloc_tile_pool(name="wp1", bufs=3)
            ab_pool = tc.alloc_tile_pool(name="abp", bufs=2)
            v_pool = tc.alloc_tile_pool(name="vp", bufs=2)
            ps_pool = tc.alloc_tile_pool(name="psp", bufs=2, space="PSUM")
            sp_pool = tc.alloc_tile_pool(name="spp", bufs=4, space="PSUM")
            yp_pool = tc.alloc_tile_pool(name="ypp", bufs=2, space="PSUM")

            # startup-critical DMA order: first w-tile, then the first
            # halves of all x tiles (qkv(h0) sweeps ct 0..15 per quarter),
            # rope tables behind the half-0 x loads, constants late.
            w0 = w_pool.tile([128, C], BF16, name="wt")
            nc.sync.dma_start(w0[:], Wall[0])
            xs = [x_pool.tile([128, 2, T], BF16, name=f"x{i}")
                  for i in range(8)]
            engs = [nc.sync, nc.scalar, nc.gpsimd]
            for q in range(2):
                for i in range(8):
                    engs[i % 3].dma_start(
                        xs[i][:, :, q * 512:(q + 1) * 512],
                        xT[:, 2 * i:2 * i + 2, q * 512:(q + 1) * 512])
            c2_sb = cpool.tile([128, T], F32)
            nc.gpsimd.dma_start(c2_sb[:], cos2)
            s1_sb = cpool.tile([64, T], F32)
            nc.scalar.dma_start(s1_sb[:], sin1)
            for q in range(2, 4):
                for i in range(8):
                    engs[i % 3].dma_start(
                        xs[i][:, :, q * 512:(q + 1) * 512],
                        xT[:, 2 * i:2 * i + 2, q * 512:(q + 1) * 512])
            m4_sb = cpool.tile([128, 4, 512], BF16)
            nc.sync.dma_start(m4_sb[:], mask4)
            id_sb = cpool.tile([128, 128], BF16)
            nc.sync.dma_start(id_sb[:], ident)
            sg_sb = cpool.tile([128, 1], F32)
            nc.sync.dma_start(sg_sb[:], sgn)

            yts = [yts_pool.tile([128, NTT, 128], BF16, name=f"yts{h}")
                   for h in range(HL)]

            # persistent double-buffered per-head q/k and v slots
            qk_sb = [qk_pool.tile([128, 2, T], BF16, name=f"qk{s}")
                     for s in range(2)]
            va_sb = [va_pool.tile([128, NTT, 129], BF16, name=f"va{s}")
                     for s in range(2)]
            for s in range(2):
                nc.vector.memset(va_sb[s][:, :, 128:129], 1.0)

            # ---------- qkv unit: one (w-kind, 512-t quarter) of head h ----------
            def units_qkv(h):
                slot = h % 2
                st = {"pend": None}
                if h == 0:
                    st[0] = w0
                units = []

                def flush_pend():
                    if st["pend"] is not None:
                        vb, ts = st["pend"]
                        for qq in range(4):
                            vtp = ps_pool.tile([128, 128], BF16,
                                               name="vtp", tag="qkvps")
                            nc.tensor.transpose(
                                vtp[:], vb[:, qq * 128:(qq + 1) * 128],
                                id_sb[:])
                            nc.scalar.copy(
                                va_sb[slot][:, 4 * ts + qq, 0:128], vtp[:])
                        st["pend"] = None

                def unit(wi, ts):
                    def go():
                        if ts == 0 and (h, wi) != (0, 0):
                            wt = w_pool.tile([128, C], BF16, name="wt")
                            eng = nc.sync if (3 * h + wi) % 2 == 0 else nc.scalar
                            eng.dma_start(wt[:], Wall[3 * h + wi])
                            st[wi] = wt
                        wt = st[wi]
                        t0 = ts * 512
                        ps = ps_pool.tile([128, 512], F32, name="qkvps")
                        for ct in range(NCT):
                            nc.tensor.matmul(
                                ps[:], wt[:, ct * 128:(ct + 1) * 128],
                                xs[ct // 2][:, ct % 2, t0:t0 + 512],
                                start=(ct == 0), stop=(ct == NCT - 1))
                        flush_pend()
                        if wi == 0:   # v -> bf16; transposes deferred
                            vb = v_pool.tile([128, 512], BF16, name="vb")
                            nc.scalar.copy(vb[:], ps[:])
                            st["pend"] = (vb, ts)
                        else:         # q/k -> rope -> persistent slot
                            a_t = ab_pool.tile([128, 512], F32, name="a_t")
                            nc.vector.tensor_mul(
                                a_t[:], ps[:], c2_sb[:, t0:t0 + 512])
                            b_t = ab_pool.tile([128, 512], F32, name="b_t")
                            nc.vector.tensor_mul(
                                b_t[0:64, :], ps[64:128, :],
                                s1_sb[:, t0:t0 + 512])
                            nc.vector.tensor_mul(
                                b_t[64:128, :], ps[0:64, :],
                                s1_sb[:, t0:t0 + 512])
                            nc.vector.scalar_tensor_tensor(
                                qk_sb[slot][:, wi - 1, t0:t0 + 512],
                                b_t[:], sg_sb[:], a_t[:],
                                op0=ALU.mult, op1=ALU.add)
                    return go

                for wi in range(3):
                    for ts in range(4):
                        units.append(unit(wi, ts))
                return units

            # ---------- attention units of head h ----------
            def units_attn(h):
                slot = h % 2
                st = {"ebs": [], "fin": None}
                units = []

                def scores(Q, su):
                    # sub-unit su covers k-tiles j = 4su .. 4su+3 of chunk Q.
                    # Diagonal tiles (su == Q) only need q >= 128*jj: attnv
                    # for q-tile ql reads eb[4Q+jj][:, ql*128:] with ql >= jj,
                    # so the strictly-above-diagonal q-range is never read.
                    def go():
                        ebs = st["ebs"]
                        if su == 0:
                            del ebs[:]
                        for jj in range(4):
                            j = 4 * su + jj
                            s0 = 128 * jj if su == Q else 0
                            sp = sp_pool.tile([128, 512], F32, name="sp")
                            nc.tensor.matmul(
                                sp[:, s0:512],
                                qk_sb[slot][:, 1, j * 128:(j + 1) * 128],
                                qk_sb[slot][:, 0,
                                            Q * 512 + s0:(Q + 1) * 512],
                                start=True, stop=True)
                            eb = eb_pool.tile([128, 512], BF16, name="eb")
                            nc.scalar.activation(
                                eb[:, s0:512], sp[:, s0:512],
                                AF.Exp, scale=float(SCALE))
                            if su == Q:   # diagonal group: causal mask
                                nc.gpsimd.tensor_mul(
                                    eb[:, s0:512], eb[:, s0:512],
                                    m4_sb[:, jj, s0:512])
                            ebs.append(eb)
                    return go

                def fin_flush():
                    if st["fin"] is not None:
                        yp, qt_i = st["fin"]
                        rc = rc_pool.tile([128, 1], F32, name="rc")
                        nc.vector.reciprocal(rc[:], yp[:, 128:129])
                        yn = yn_pool.tile([128, 128], BF16, name="yn")
                        nc.vector.tensor_scalar_mul(yn[:], yp[:, 0:128], rc[:])
                        ytp = yp_pool.tile([128, 128], BF16, name="ytp",
                                           tag="yp")
                        nc.tensor.transpose(ytp[:], yn[:], id_sb[:])
                        nc.vector.tensor_copy(yts[h][:, qt_i, :], ytp[:])
                        st["fin"] = None

                def attnv(Q, ql):
                    def go():
                        ebs = st["ebs"]
                        qt_i = Q * 4 + ql
                        yp = yp_pool.tile([128, 129], F32, name="yp")
                        for j in range(qt_i + 1):
                            nc.tensor.matmul(
                                yp[:],
                                ebs[j][:, ql * 128:(ql + 1) * 128],
                                va_sb[slot][:, j, :],
                                start=(j == 0), stop=(j == qt_i))
                        fin_flush()
                        st["fin"] = (yp, qt_i)
                    return go

                def epi():
                    def go():
                        fin_flush()
                        nc.gpsimd.dma_start(
                            yg_in[h].rearrange("d (tt t) -> d tt t", t=128),
                            yts[h][:])
                        nc.gpsimd.collective_compute(
                            "AllGather", ALU.bypass,
                            ins=[yg_in[h][:].opt()],
                            outs=[yg_out[h][:].opt()],
                            replica_groups=RG)
                    return go

                for Q in range(4):
                    for su in range(Q + 1):
                        units.append(scores(Q, su))
                    for ql in range(4):
                        units.append(attnv(Q, ql))
                units.append(epi())
                return units

            # ---------- emit qkv + attention for heads 0..6 ----------
            for u in units_qkv(0):
                u()
            for h in range(HL - 1):
                nxt = units_qkv(h + 1) if h + 1 < HL else []
                _interleave(units_attn(h), nxt)

            # qkv all done: free x/w/rope space, prefetch proj inputs.
            # pass-1a pp tiles reuse the qkv PSUM ring (tag "qkvps").
            for pool in [v_pool, ab_pool, w_pool, x_pool]:
                pool.release()
            wp_pool = tc.alloc_tile_pool(name="p4w", bufs=1)
            yf_pool = tc.alloc_tile_pool(name="p4y", bufs=1)
            o_pool = tc.alloc_tile_pool(name="p4o", bufs=1)
            wp = wp_pool.tile([128, NCT, C // 2], BF16)
            yfs = {}
            for h in range(4):
                yf = yf_pool.tile([128, 2, NTT, 128], BF16, name="yf",
                                  bufs=4)
                nc.scalar.dma_start(
                    yf[:], yg_out[h][:].rearrange("r d (tt t) -> d r tt t",
                                                  t=128))
                yfs[h] = yf
            nc.sync.dma_start(wp[:, :, 0:512], WpT[:, :, 0:512])
            nc.sync.dma_start(wp[:, :, 512:1024], WpT[:, :, 512:1024])
            obs = [o_pool.tile([128, 512], BF16, name=f"ob{i}")
                   for i in range(32)]

            def proj_units(heads, pi, pool, tag, o2_pool=None):
                cts = [(r, h) for h in heads for r in range(2)]
                units = []

                def group(tt, fc):
                    def go():
                        ob = obs[tt * 2 + fc]
                        pp = pool.tile([128, 512], F32, name="pp", tag=tag)
                        for i, (r, h) in enumerate(cts):
                            nc.tensor.matmul(
                                pp[:], yfs[h][:, r, tt, :],
                                wp[:, r * 8 + h, fc * 512:(fc + 1) * 512],
                                start=(i == 0), stop=(i == len(cts) - 1))
                        if pi == 0:
                            nc.vector.tensor_copy(ob[:], pp[:])
                        elif o2_pool is None:
                            nc.vector.tensor_add(ob[:], ob[:], pp[:])
                        else:
                            o2 = o2_pool.tile([128, 512], F32, name="o2")
                            nc.vector.tensor_add(o2[:], ob[:], pp[:])
                            oeng = [nc.sync, nc.scalar, nc.gpsimd][
                                (tt * 2 + fc) % 3]
                            oeng.dma_start(
                                out[tt * 128:(tt + 1) * 128,
                                    fc * 512:(fc + 1) * 512], o2[:])
                    return go

                for fc in range(2):
                    for tt in range(NTT):
                        units.append(group(tt, fc))
                return units

            # last head's attention interleaved with proj pass over h0-3
            attn7 = units_attn(7)
            p1a = proj_units([0, 1, 2, 3], 0, ps_pool, "qkvps")
            for u in attn7[:18]:
                u()
            _interleave(attn7[18:], p1a)

            for pool in [yp_pool, sp_pool]:
                pool.release()
            pp_b = tc.alloc_tile_pool(name="ppb", bufs=6, space="PSUM")
            o2_pool = tc.alloc_tile_pool(name="p4o2", bufs=4)
            for h in range(4, 8):
                yf = yf_pool.tile([128, 2, NTT, 128], BF16, name="yf",
                                  bufs=4)
                nc.scalar.dma_start(
                    yf[:], yg_out[h][:].rearrange("r d (tt t) -> d r tt t",
                                                  t=128))
                yfs[h] = yf
            for u in proj_units([4, 5], 1, pp_b, "ppb"):
                u()
            for u in proj_units([6], 2, pp_b, "ppb"):
                u()
            for u in proj_units([7], 3, pp_b, "ppb", o2_pool=o2_pool):
                u()
            for pool in [o2_pool, pp_b, o_pool, yf_pool, wp_pool,
                         rc_pool, yn_pool, eb_pool, va_pool, qk_pool,
                         yts_pool, ps_pool]:
                pool.release()
    nc.compile()
    return nc


_NC = None


def _get_nc():
    global _NC
    if _NC is None:
        _NC = _build()
    return _NC


def _rope_tables():
    inv_freq = (1.0 / (10000.0 ** (np.arange(0, D, 2, dtype=np.float32) / D)))
    t = np.arange(T, dtype=np.float32)
    freqs = np.outer(t, inv_freq).astype(np.float32)      # [T, 64]
    cos = np.cos(freqs).T                                 # [64, T]
    sin = np.sin(freqs).T
    cos2 = np.concatenate([cos, cos], 0).astype(np.float32)
    sin1 = np.ascontiguousarray(sin.astype(np.float32))
    return cos2, sin1


def _tile_w(Wt):
    """[128 r, 2048 c] weight tile -> [128 c_lo, 2048 (ct r)] layout."""
    return np.ascontiguousarray(
        Wt.T.reshape(NCT, 128, 128).transpose(1, 0, 2).reshape(128, C))


def make_in_maps(x, W_attn, W_proj):
    perm = np.concatenate([np.arange(0, D, 2), np.arange(1, D, 2)])
    cos2, sin1 = _rope_tables()
    sgn = np.concatenate([-np.ones((64, 1)), np.ones((64, 1))]).astype(np.float32)
    p_i = np.arange(128)[:, None, None]
    jj_i = np.arange(4)[None, :, None]
    c_i = np.arange(512)[None, None, :]
    mask4 = (c_i >= p_i + 128 * jj_i).astype(BF16NP)

    in_maps = []
    for core in range(8):
        b, g = core // 2, core % 2
        tiles = []
        for h in range(HL):
            hg = g * HL + h
            tiles.append(_tile_w(W_attn[2 * C + hg * D:2 * C + (hg + 1) * D]))
            tiles.append(_tile_w(W_attn[hg * D:(hg + 1) * D][perm]))
            tiles.append(_tile_w(W_attn[C + hg * D:C + (hg + 1) * D][perm]))
        Wall = np.stack(tiles, 0).astype(BF16NP)
        WpT = np.ascontiguousarray(
            W_proj[g * (C // 2):(g + 1) * (C // 2), :].T
        ).reshape(NCT, 128, C // 2).transpose(1, 0, 2)
        xTc = np.ascontiguousarray(
            x[b].T.reshape(NCT, 128, T).transpose(1, 0, 2)).astype(BF16NP)
        in_maps.append({
            "xT": xTc,
            "Wall": Wall,
            "WpT": np.ascontiguousarray(WpT).astype(BF16NP),
            "cos2": cos2, "sin1": sin1, "sgn": sgn,
            "mask4": mask4, "ident": np.eye(128, dtype=BF16NP),
        })
    return in_maps


def _assemble(results):
    out = np.empty((B, T, C), dtype=np.float32)
    for core in range(8):
        b, g = core // 2, core % 2
        out[b][:, g * (C // 2):(g + 1) * (C // 2)] = results[core]["out"]
    return out


def run(x, W_attn, W_proj, **spmd_kwargs):
    nc = _get_nc()
    in_maps = make_in_maps(np.asarray(x, dtype=np.float32),
                           np.asarray(W_attn, dtype=np.float32),
                           np.asarray(W_proj, dtype=np.float32))
    res = bass_utils.run_bass_kernel_spmd(
        nc, in_maps, core_ids=list(range(8)), **spmd_kwargs)
    return _assemble(res.results), res


def kernel(x, W_attn, W_proj):
    out, _ = run(x, W_attn, W_proj)
    return out


# revision 30
# speedup vs baseline: 1.1156x; 1.1156x over previous
"""Causal self-attention (B=4, T=2048, C=2048, H=16, rope) on 8 trn2 cores.

Sharding: core c handles batch b = c//2 and head-group g = c%2 (8 heads).

All-bf16 datapath (PSUM accumulation in f32): x is host-cast to bf16 and
kept fully resident in SBUF; q/k/v never leave SBUF (no DRAM round-trip).
Rolling per-head schedule: attention of head h is emission-interleaved
with the qkv GEMMs of head h+1, so the ACT-bound softmax hides under the
PE-bound qkv and each head's y AllGather fires at head cadence.

  qkv (per head, per 512-t quarter): ps = W_tile^T x (16 bf16 matmuls,
    N=512); v is cast bf16 + PE-transposed into va[t,d] with a ones
    column for the softmax denominator (transposes deferred one unit so
    the ACT cast never stalls the PE); q/k get rope on DVE (sign-vector
    trick) and land bf16 in a persistent per-head SBUF slot.
  attention (per head, per 512-q chunk): scoresT = k_tile^T q into
    single-bank PSUM tiles (4 bufs) so exp on ACT pipelines 4 deep,
    causal mask multiply on Pool, attn@V with the ones column so the
    denominator falls out of the same matmul, reciprocal normalize,
    PE-transpose y to [d, t] (deferred one q-tile).
  epilogue (per head): yts -> DRAM on the Pool queue, then pairwise
    AllGather (8 small collectives at head cadence).
  proj: out[t, f-half] accumulated in four passes by collective arrival
    time (heads 0-3 / 4-5 / 6 / 7) with bf16 partials resident in SBUF;
    the first pass is interleaved into the last head's ACT-bound
    attention, and the late pair-core collectives never stall the PE.
"""
import sys

sys.path.insert(0, "/opt/trn_rl_repo")

import numpy as np
import ml_dtypes

import concourse.bass as bass
import concourse.tile as tile
from concourse import bacc, mybir
from concourse import bass_utils

F32 = mybir.dt.float32
BF16 = mybir.dt.bfloat16
AF = mybir.ActivationFunctionType
ALU = mybir.AluOpType
BF16NP = ml_dtypes.bfloat16

B, T, C = 4, 2048, 2048
NH, D = 16, 128
HL = 8              # heads per core
NCT = C // 128      # 16 c-tiles
NTT = T // 128      # 16 t-tiles
SCALE = 1.0 / np.sqrt(D)
RG = [[0, 1], [2, 3], [4, 5], [6, 7]]


def _interleave(units_a, units_b):
    """Round-robin emit closures from two lists, proportionally."""
    na, nb = len(units_a), len(units_b)
    ia = ib = 0
    while ia < na or ib < nb:
        if ib >= nb or (ia < na and ia * nb <= ib * na):
            units_a[ia]()
            ia += 1
        else:
            units_b[ib]()
            ib += 1


def _build():
    nc = bacc.Bacc("TRN2", target_bir_lowering=False, debug=False, num_devices=8)
    xT = nc.dram_tensor("xT", [128, NCT, T], BF16, kind="ExternalInput").ap()
    Wall = nc.dram_tensor("Wall", [24, 128, C], BF16, kind="ExternalInput").ap()
    WpT = nc.dram_tensor("WpT", [128, NCT, C // 2], BF16, kind="ExternalInput").ap()
    cos2 = nc.dram_tensor("cos2", [128, T], BF16, kind="ExternalInput").ap()
    sin1 = nc.dram_tensor("sin1", [64, T], BF16, kind="ExternalInput").ap()
    sgn = nc.dram_tensor("sgn", [128, 1], F32, kind="ExternalInput").ap()
    mask4 = nc.dram_tensor("mask4", [128, 4, 512], BF16, kind="ExternalInput").ap()
    ident = nc.dram_tensor("ident", [128, 128], BF16, kind="ExternalInput").ap()
    out = nc.dram_tensor("out", [T, C // 2], F32, kind="ExternalOutput").ap()

    with tile.TileContext(nc) as tc:
        with tc.tile_pool(name="dram", bufs=1, space="DRAM") as dram, \
             tc.tile_pool(name="const", bufs=1) as cpool:
            yg_in = [dram.tile([128, T], BF16, name=f"yg_in{h}")
                     for h in range(HL)]
            yg_out = [dram.tile([2, 128, T], BF16, name=f"yg_out{h}")
                      for h in range(HL)]

            # stack order matters: pools released early must be on top
            wp_pool = tc.alloc_tile_pool(name="p4w", bufs=1)
            yts_pool = tc.alloc_tile_pool(name="ytsp", bufs=1)
            qk_pool = tc.alloc_tile_pool(name="qkp", bufs=1)
            va_pool = tc.alloc_tile_pool(name="vap", bufs=1)
            eb_pool = tc.alloc_tile_pool(name="ebp", bufs=20)
            yn_pool = tc.alloc_tile_pool(name="ynp", bufs=3)
            rc_pool = tc.alloc_tile_pool(name="rcp", bufs=3)
            x_pool = tc.alloc_tile_pool(name="xp", bufs=1)
            w_pool = tc.alloc_tile_pool(name="wp1", bufs=2)
            ab_pool = tc.alloc_tile_pool(name="abp", bufs=1)
            v_pool = tc.alloc_tile_pool(name="vp", bufs=2)
            ps_pool = tc.alloc_tile_pool(name="psp", bufs=2, space="PSUM")
            sp_pool = tc.alloc_tile_pool(name="spp", bufs=4, space="PSUM")
            yp_pool = tc.alloc_tile_pool(name="ypp", bufs=2, space="PSUM")

            # startup-critical DMA order: first w-tile, then the first
            # halves of all x tiles (qkv(h0) sweeps ct 0..15 per quarter),
            # rope tables behind the half-0 x loads, constants late.
            w0 = w_pool.tile([128, C], BF16, name="wt")
            nc.sync.dma_start(w0[:], Wall[0])
            xs = [x_pool.tile([128, 2, T], BF16, name=f"x{i}")
                  for i in range(8)]
            engs = [nc.sync, nc.scalar, nc.gpsimd]
            for q in range(2):
                for i in range(8):
                    engs[i % 3].dma_start(
                        xs[i][:, :, q * 512:(q + 1) * 512],
                        xT[:, 2 * i:2 * i + 2, q * 512:(q + 1) * 512])
            c2_sb = cpool.tile([128, T], BF16)
            nc.gpsimd.dma_start(c2_sb[:], cos2)
            s1_sb = cpool.tile([64, T], BF16)
            nc.scalar.dma_start(s1_sb[:], sin1)
            for q in range(2, 4):
                for i in range(8):
                    engs[i % 3].dma_start(
                        xs[i][:, :, q * 512:(q + 1) * 512],
                        xT[:, 2 * i:2 * i + 2, q * 512:(q + 1) * 512])
            m4_sb = cpool.tile([128, 4, 512], BF16)
            nc.sync.dma_start(m4_sb[:], mask4)
            id_sb = cpool.tile([128, 128], BF16)
            nc.sync.dma_start(id_sb[:], ident)
            sg_sb = cpool.tile([128, 1], F32)
            nc.sync.dma_start(sg_sb[:], sgn)

            yts = [yts_pool.tile([128, NTT, 128], BF16, name=f"yts{h}")
                   for h in range(HL)]

            # persistent double-buffered per-head q/k and v slots
            qk_sb = [qk_pool.tile([128, 2, T], BF16, name=f"qk{s}")
                     for s in range(2)]
            va_sb = [va_pool.tile([128, NTT, 129], BF16, name=f"va{s}")
                     for s in range(2)]
            for s in range(2):
                nc.vector.memset(va_sb[s][:, :, 128:129], 1.0)

            # ---------- qkv unit: one (w-kind, 512-t quarter) of head h ----------
            def units_qkv(h):
                slot = h % 2
                st = {"pend": None}
                if h == 0:
                    st[0] = w0
                units = []

                def flush_pend():
                    if st["pend"] is not None:
                        vb, ts = st["pend"]
                        for qq in range(4):
                            vtp = ps_pool.tile([128, 128], BF16,
                                               name="vtp", tag="qkvps")
                            nc.tensor.transpose(
                                vtp[:], vb[:, qq * 128:(qq + 1) * 128],
                                id_sb[:])
                            nc.scalar.copy(
                                va_sb[slot][:, 4 * ts + qq, 0:128], vtp[:])
                        st["pend"] = None

                def unit(wi, ts):
                    def go():
                        if ts == 0 and (h, wi) != (0, 0):
                            wt = w_pool.tile([128, C], BF16, name="wt")
                            eng = nc.sync if (3 * h + wi) % 2 == 0 else nc.scalar
                            eng.dma_start(wt[:], Wall[3 * h + wi])
                            st[wi] = wt
                        wt = st[wi]
                        t0 = ts * 512
                        ps = ps_pool.tile([128, 512], F32, name="qkvps")
                        for ct in range(NCT):
                            nc.tensor.matmul(
                                ps[:], wt[:, ct * 128:(ct + 1) * 128],
                                xs[ct // 2][:, ct % 2, t0:t0 + 512],
                                start=(ct == 0), stop=(ct == NCT - 1))
                        flush_pend()
                        if wi == 0:   # v -> bf16; transposes deferred
                            vb = v_pool.tile([128, 512], BF16, name="vb")
                            nc.scalar.copy(vb[:], ps[:])
                            st["pend"] = (vb, ts)
                        else:         # q/k -> rope -> persistent slot
                            a_t = ab_pool.tile([128, 512], F32, name="a_t")
                            nc.vector.tensor_mul(
                                a_t[:], ps[:], c2_sb[:, t0:t0 + 512])
                            b_t = ab_pool.tile([128, 512], F32, name="b_t")
                            nc.vector.tensor_mul(
                                b_t[0:64, :], ps[64:128, :],
                                s1_sb[:, t0:t0 + 512])
                            nc.vector.tensor_mul(
                                b_t[64:128, :], ps[0:64, :],
                                s1_sb[:, t0:t0 + 512])
                            nc.vector.scalar_tensor_tensor(
                                qk_sb[slot][:, wi - 1, t0:t0 + 512],
                                b_t[:], sg_sb[:], a_t[:],
                                op0=ALU.mult, op1=ALU.add)
                    return go

                for wi in range(3):
                    for ts in range(4):
                        units.append(unit(wi, ts))
                return units

            # ---------- attention units of head h ----------
            def units_attn(h):
                slot = h % 2
                st = {"ebs": [], "fin": None}
                units = []

                def scores(Q, su):
                    # sub-unit su covers k-tiles j = 4su .. 4su+3 of chunk Q.
                    # Diagonal tiles (su == Q) only need q >= 128*jj: attnv
                    # for q-tile ql reads eb[4Q+jj][:, ql*128:] with ql >= jj,
                    # so the strictly-above-diagonal q-range is never read.
                    def go():
                        ebs = st["ebs"]
                        if su == 0:
                            del ebs[:]
                        for jj in range(4):
                            j = 4 * su + jj
                            s0 = 128 * jj if su == Q else 0
                            sp = sp_pool.tile([128, 512], F32, name="sp")
                            nc.tensor.matmul(
                                sp[:, s0:512],
                                qk_sb[slot][:, 1, j * 128:(j + 1) * 128],
                                qk_sb[slot][:, 0,
                                            Q * 512 + s0:(Q + 1) * 512],
                                start=True, stop=True)
                            eb = eb_pool.tile([128, 512], BF16, name="eb")
                            nc.scalar.activation(
                                eb[:, s0:512], sp[:, s0:512],
                                AF.Exp, scale=float(SCALE))
                            if su == Q:   # diagonal group: causal mask
                                nc.gpsimd.tensor_mul(
                                    eb[:, s0:512], eb[:, s0:512],
                                    m4_sb[:, jj, s0:512])
                            ebs.append(eb)
                    return go

                def fin_flush():
                    if st["fin"] is not None:
                        yp, qt_i = st["fin"]
                        rc = rc_pool.tile([128, 1], F32, name="rc")
                        nc.vector.reciprocal(rc[:], yp[:, 128:129])
                        yn = yn_pool.tile([128, 128], BF16, name="yn")
                        nc.vector.tensor_scalar_mul(yn[:], yp[:, 0:128], rc[:])
                        ytp = yp_pool.tile([128, 128], BF16, name="ytp",
                                           tag="yp")
                        nc.tensor.transpose(ytp[:], yn[:], id_sb[:])
                        nc.vector.tensor_copy(yts[h][:, qt_i, :], ytp[:])
                        st["fin"] = None

                def attnv(Q, ql):
                    def go():
                        ebs = st["ebs"]
                        qt_i = Q * 4 + ql
                        yp = yp_pool.tile([128, 129], F32, name="yp")
                        for j in range(qt_i + 1):
                            nc.tensor.matmul(
                                yp[:],
                                ebs[j][:, ql * 128:(ql + 1) * 128],
                                va_sb[slot][:, j, :],
                                start=(j == 0), stop=(j == qt_i))
                        fin_flush()
                        st["fin"] = (yp, qt_i)
                    return go

                def epi():
                    def go():
                        fin_flush()
                        nc.gpsimd.dma_start(
                            yg_in[h].rearrange("d (tt t) -> d tt t", t=128),
                            yts[h][:])
                        nc.gpsimd.collective_compute(
                            "AllGather", ALU.bypass,
                            ins=[yg_in[h][:].opt()],
                            outs=[yg_out[h][:].opt()],
                            replica_groups=RG)
                    return go

                for Q in range(4):
                    for su in range(Q + 1):
                        units.append(scores(Q, su))
                    for ql in range(4):
                        units.append(attnv(Q, ql))
                units.append(epi())
                return units

            # ---------- emit qkv + attention for heads 0..6 ----------
            # wp lives in a long-lived slot, loaded early on the idle
            # Pool DMA queue so the proj passes never wait for it.
            wp = wp_pool.tile([128, NCT, C // 2], BF16)
            for u in units_qkv(0):
                u()
            _interleave(units_attn(0), units_qkv(1))
            nc.gpsimd.dma_start(wp[:, :, 0:512], WpT[:, :, 0:512])
            nc.gpsimd.dma_start(wp[:, :, 512:1024], WpT[:, :, 512:1024])
            for h in range(1, HL - 1):
                nxt = units_qkv(h + 1) if h + 1 < HL else []
                _interleave(units_attn(h), nxt)

            # qkv all done: free x/w/rope space, prefetch proj inputs.
            # pass-1a pp tiles reuse the qkv PSUM ring (tag "qkvps").
            for pool in [v_pool, ab_pool, w_pool, x_pool]:
                pool.release()
            yf_pool = tc.alloc_tile_pool(name="p4y", bufs=1)
            o_pool = tc.alloc_tile_pool(name="p4o", bufs=1)
            yfs = {}
            for h in range(4):
                yf = yf_pool.tile([128, 2, NTT, 128], BF16, name="yf",
                                  bufs=4)
                nc.scalar.dma_start(
                    yf[:], yg_out[h][:].rearrange("r d (tt t) -> d r tt t",
                                                  t=128))
                yfs[h] = yf
            obs = [o_pool.tile([128, 512], BF16, name=f"ob{i}")
                   for i in range(32)]

            def proj_units(heads, pi, pool, tag, o2_pool=None):
                cts = [(r, h) for h in heads for r in range(2)]
                units = []

                def group(tt, fc):
                    def go():
                        ob = obs[tt * 2 + fc]
                        pp = pool.tile([128, 512], F32, name="pp", tag=tag)
                        for i, (r, h) in enumerate(cts):
                            nc.tensor.matmul(
                                pp[:], yfs[h][:, r, tt, :],
                                wp[:, r * 8 + h, fc * 512:(fc + 1) * 512],
                                start=(i == 0), stop=(i == len(cts) - 1))
                        if pi == 0:
                            nc.vector.tensor_copy(ob[:], pp[:])
                        elif o2_pool is None:
                            nc.vector.tensor_add(ob[:], ob[:], pp[:])
                        else:
                            o2 = o2_pool.tile([128, 512], F32, name="o2")
                            nc.vector.tensor_add(o2[:], ob[:], pp[:])
                            oeng = [nc.sync, nc.scalar, nc.gpsimd][
                                (tt * 2 + fc) % 3]
                            oeng.dma_start(
                                out[tt * 128:(tt + 1) * 128,
                                    fc * 512:(fc + 1) * 512], o2[:])
                    return go

                for fc in range(2):
                    for tt in range(NTT):
                        units.append(group(tt, fc))
                return units

            # last head's attention interleaved with proj pass over h0-3
            attn7 = units_attn(7)
            p1a = proj_units([0, 1], 0, ps_pool, "qkvps")
            for u in attn7[:18]:
                u()
            _interleave(attn7[18:], p1a)

            for pool in [yp_pool, sp_pool]:
                pool.release()
            pp_b = tc.alloc_tile_pool(name="ppb", bufs=6, space="PSUM")
            o2_pool = tc.alloc_tile_pool(name="p4o2", bufs=4)
            for h in range(4, 8):
                yf = yf_pool.tile([128, 2, NTT, 128], BF16, name="yf",
                                  bufs=4)
                nc.scalar.dma_start(
                    yf[:], yg_out[h][:].rearrange("r d (tt t) -> d r tt t",
                                                  t=128))
                yfs[h] = yf
            for u in proj_units([2, 3], 1, pp_b, "ppb"):
                u()
            for u in proj_units([4, 5], 2, pp_b, "ppb"):
                u()
            for u in proj_units([6], 3, pp_b, "ppb"):
                u()
            for u in proj_units([7], 4, pp_b, "ppb", o2_pool=o2_pool):
                u()
            for pool in [o2_pool, pp_b, o_pool, yf_pool,
                         rc_pool, yn_pool, eb_pool, va_pool, qk_pool,
                         yts_pool, wp_pool, ps_pool]:
                pool.release()
    nc.compile()
    return nc


_NC = None


def _get_nc():
    global _NC
    if _NC is None:
        _NC = _build()
    return _NC


def _rope_tables():
    inv_freq = (1.0 / (10000.0 ** (np.arange(0, D, 2, dtype=np.float32) / D)))
    t = np.arange(T, dtype=np.float32)
    freqs = np.outer(t, inv_freq).astype(np.float32)      # [T, 64]
    cos = np.cos(freqs).T                                 # [64, T]
    sin = np.sin(freqs).T
    cos2 = np.concatenate([cos, cos], 0).astype(BF16NP)
    sin1 = np.ascontiguousarray(sin.astype(BF16NP))
    return cos2, sin1


def _tile_w(Wt):
    """[128 r, 2048 c] weight tile -> [128 c_lo, 2048 (ct r)] layout."""
    return np.ascontiguousarray(
        Wt.T.reshape(NCT, 128, 128).transpose(1, 0, 2).reshape(128, C))


def make_in_maps(x, W_attn, W_proj):
    perm = np.concatenate([np.arange(0, D, 2), np.arange(1, D, 2)])
    cos2, sin1 = _rope_tables()
    sgn = np.concatenate([-np.ones((64, 1)), np.ones((64, 1))]).astype(np.float32)
    p_i = np.arange(128)[:, None, None]
    jj_i = np.arange(4)[None, :, None]
    c_i = np.arange(512)[None, None, :]
    mask4 = (c_i >= p_i + 128 * jj_i).astype(BF16NP)

    in_maps = []
    for core in range(8):
        b, g = core // 2, core % 2
        tiles = []
        for h in range(HL):
            hg = g * HL + h
            tiles.append(_tile_w(W_attn[2 * C + hg * D:2 * C + (hg + 1) * D]))
            tiles.append(_tile_w(W_attn[hg * D:(hg + 1) * D][perm]))
            tiles.append(_tile_w(W_attn[C + hg * D:C + (hg + 1) * D][perm]))
        Wall = np.stack(tiles, 0).astype(BF16NP)
        WpT = np.ascontiguousarray(
            W_proj[g * (C // 2):(g + 1) * (C // 2), :].T
        ).reshape(NCT, 128, C // 2).transpose(1, 0, 2)
        xTc = np.ascontiguousarray(
            x[b].T.reshape(NCT, 128, T).transpose(1, 0, 2)).astype(BF16NP)
        in_maps.append({
            "xT": xTc,
            "Wall": Wall,
            "WpT": np.ascontiguousarray(WpT).astype(BF16NP),
            "cos2": cos2, "sin1": sin1, "sgn": sgn,
            "mask4": mask4, "ident": np.eye(128, dtype=BF16NP),
        })
    return in_maps


def _assemble(results):
    out = np.empty((B, T, C), dtype=np.float32)
    for core in range(8):
        b, g = core // 2, core % 2
        out[b][:, g * (C // 2):(g + 1) * (C // 2)] = results[core]["out"]
    return out


def run(x, W_attn, W_proj, **spmd_kwargs):
    nc = _get_nc()
    in_maps = make_in_maps(np.asarray(x, dtype=np.float32),
                           np.asarray(W_attn, dtype=np.float32),
                           np.asarray(W_proj, dtype=np.float32))
    res = bass_utils.run_bass_kernel_spmd(
        nc, in_maps, core_ids=list(range(8)), **spmd_kwargs)
    return _assemble(res.results), res


def kernel(x, W_attn, W_proj):
    out, _ = run(x, W_attn, W_proj)
    return out
